# revision 20
# baseline (speedup 1.0000x reference)
"""Multi-head attention (B=2, L=S=2048, D=1024, H=16) on 8 Trainium2 cores.

Sharding: core c -> batch b = c // 4, head group g = c % 4 (4 heads per core).
W_Q/K/V column-sharded (256 cols per core), W_O row-sharded (256 rows per core);
the 4 partial outputs per batch are summed on the host (plus bias terms).

Per-core pipeline (all big tensors kept transposed so no on-device transposes):
  projections: QT = 0.125*(x Wq + bq)^T, KT = (x Wk + bk)^T (feature-major
    [256, L]); Vaug = [V_h | ones] per head (seq-major, fp16), V bias folded
    out on the host (softmax rows sum to 1 => + bv @ Wo + bo once).
  attention, per (l-tile 512, s-tile 128): S^T = KT^T QT (row-packed pairs of
    heads, K=64, the two 64-row matmuls co-execute in disjoint PE row
    halves); E = exp(S^T) * maskT; T_h += Vaug_h^T E accumulates the head
    output AND its softmax row-sums in one matmul (ones columns act as the
    reducer).

Scheduling: the 64 (lt, st) attention tiles form ONE flat software pipeline
with the AV accumulation running four s-tiles behind the score matmuls -
including across lt boundaries - so the PE never idle-waits on the
ACT exp -> DVE mask chain.  Idle waits re-throttle the PE clock to 1.2 GHz
(HAM activity monitor); in the unpipelined version ~36% of matmuls ran at
half clock.  The QT projection chain for lt+1 is emitted in two halves
mid-lt and each lt's out-projection is spread through the NEXT lt (PE
filler during the ACT-bound steady state); only lt3's out-projection runs
as a tail.  x arrives via one coarse strided DMA per l-quarter so KT/QT
chunk 0 finish after ~25% of x has landed and the exp pipeline starts
early.  ACT does exp only (+4 tail copies); QT/KT biases are DVE
scalar_tensor_tensor; softmax-sum lane swaps and output stores ride the
Sync DMA queue.

All matmul operands fp16 (1 cyc/row); PSUM fp32.  PSUM budget 8 banks =
scores 2x2 + T_h 4x1; projection and output-projection matmuls borrow the
same slots.
"""
from contextlib import ExitStack

import numpy as np

import concourse.bass as bass
import concourse.mybir as mybir
import concourse.tile as tile
from concourse import bacc
from concourse.bass_utils import run_bass_kernel_spmd

F16 = mybir.dt.float16
F32 = mybir.dt.float32
F8 = mybir.dt.float8e4
U16 = mybir.dt.uint16
DRMODE = mybir.MatmulPerfMode.DoubleRow

D = 1024          # d_model
H = 16            # heads
DK = 64           # head dim
B, L = 2, 2048
NCORES = 8
HPC = 4           # heads per core
FPC = HPC * DK    # features per core = 256
KD = D // 128     # 8 contraction subtiles for projections
LT, LTW = 4, 512  # l tiles
ST, STW = 16, 128  # s tiles
Exp = mybir.ActivationFunctionType.Exp

_CACHED_NC = None


def _build():
    nc = bacc.Bacc("TRN2", target_bir_lowering=False, debug=False,
                   num_devices=NCORES)
    xT = nc.declare_dram_parameter("xT", [128, KD, L], F16, isOutput=False)
    wq = nc.declare_dram_parameter("wq", [128, KD, FPC], F16, isOutput=False)
    wk = nc.declare_dram_parameter("wk", [128, KD, FPC], F16, isOutput=False)
    wv = nc.declare_dram_parameter("wv", [128, KD, FPC], F16, isOutput=False)
    wo = nc.declare_dram_parameter("wo", [128, 2, D], F16, isOutput=False)
    bq = nc.declare_dram_parameter("bq", [128, 2], F32, isOutput=False)
    bk = nc.declare_dram_parameter("bk", [128, 2], F32, isOutput=False)
    maskB = nc.declare_dram_parameter("maskB", [LT, 128, ST, LTW // 2], U16,
                                      isOutput=False)
    maskS8 = nc.declare_dram_parameter("maskS8", [LT, 128, 2, LTW], F16,
                                       isOutput=False)
    out = nc.declare_dram_parameter("out", [128, ST, D], F16, isOutput=True)

    with tile.TileContext(nc) as tc, ExitStack() as ctx:
        pool = ctx.enter_context(tc.tile_pool(name="pers", bufs=1))
        mpool = ctx.enter_context(tc.tile_pool(name="mpool", bufs=7))
        epool = ctx.enter_context(tc.tile_pool(name="epool", bufs=7))
        rbpool = ctx.enter_context(tc.tile_pool(name="rbpool", bufs=4))
        opool = ctx.enter_context(tc.tile_pool(name="opool", bufs=3))
        scp = ctx.enter_context(tc.tile_pool(name="scp", bufs=2, space="PSUM"))
        tp = ctx.enter_context(tc.tile_pool(name="tp", bufs=1, space="PSUM"))

        xt = pool.tile([128, KD, L], F16)
        wq_sb = pool.tile([128, KD, FPC], F16)
        wk_sb = pool.tile([128, KD, FPC], F16)
        wv_sb = pool.tile([128, KD, FPC], F16)
        wo_sb = pool.tile([128, 2, D], F16)
        bq_sb = pool.tile([128, 2], F32)
        bk_sb = pool.tile([128, 2], F32)
        # DMA order: l-quarter-major for xT (coarse strided DMAs - each
        # dma_start costs ~600ns of Sync queue time) so the KT-chunk-0 /
        # QT-chunk-0 chains (which contract over all KD but only touch
        # l 0:512) finish after ~25% of x has landed.
        nc.sync.dma_start(out=wk_sb[:], in_=wk[:])
        nc.sync.dma_start(out=xt[:, 0:4, 0:LTW], in_=xT[:, 0:4, 0:LTW])
        nc.sync.dma_start(out=xt[:, 4:KD, 0:LTW], in_=xT[:, 4:KD, 0:LTW])
        nc.sync.dma_start(out=wq_sb[:], in_=wq[:])
        nc.sync.dma_start(out=bk_sb[:], in_=bk[:])
        nc.sync.dma_start(out=bq_sb[:], in_=bq[:])

        QT = pool.tile([128, 2, L], F16)   # [feat(2x128), l]: Q^T * 0.125
        KT = pool.tile([128, 2, L], F16)
        # pair-1 heads (local h 2,3), fp16: [V_h | 1] / [1 | V_h] per parity
        Vaug = pool.tile([128, ST, 2, 128], F16)
        nc.gpsimd.memset(Vaug[:], 1.0)
        # pair-0 heads (local h 0,1), fp8 for DoubleRow AV:
        # [s, stq(8), stp(2), h(2), 128]
        Vaug8 = pool.tile([128, ST // 2, 2, 2, 128], F8)
        nc.gpsimd.memset(Vaug8[:], 1.0)
        outTs = [pool.tile([128, 2, LTW], F16, name=f"outT{i}")
                 for i in range(LT)]

        def emit_kt_chunk(c):
            lsl = slice(c * LTW, (c + 1) * LTW)
            ps = scp.tile([128, 2, LTW], F32, tag="sc", name=f"pk{c}")
            for ft in range(2):
                fsl = slice(ft * 128, (ft + 1) * 128)
                for kd in range(KD):
                    nc.tensor.matmul(ps[:, ft, :], wk_sb[:, kd, fsl],
                                     xt[:, kd, lsl],
                                     start=(kd == 0), stop=(kd == KD - 1))
                nc.vector.scalar_tensor_tensor(
                    KT[:, ft, lsl], ps[:, ft, :], 1.0,
                    bk_sb[:, ft:ft + 1].to_broadcast((128, LTW)),
                    mybir.AluOpType.mult, mybir.AluOpType.add)

        def emit_kt_half(c, ft):
            lsl = slice(c * LTW, (c + 1) * LTW)
            psk = scp.tile([128, 1, LTW], F32, tag="sc", name=f"pk{c}_{ft}")
            fsl = slice(ft * 128, (ft + 1) * 128)
            for kd in range(KD):
                nc.tensor.matmul(psk[:, 0, :], wk_sb[:, kd, fsl],
                                 xt[:, kd, lsl],
                                 start=(kd == 0), stop=(kd == KD - 1))
            nc.vector.scalar_tensor_tensor(
                KT[:, ft, lsl], psk[:, 0, :], 1.0,
                bk_sb[:, ft:ft + 1].to_broadcast((128, LTW)),
                mybir.AluOpType.mult, mybir.AluOpType.add)

        def emit_v_piece(st):
            ssl = slice(st * STW, (st + 1) * STW)
            q, p = divmod(st, 2)
            # rides the sc ring (a filler like kt/qt); T banks are taken
            # by the running lt's accumulators
            psv = scp.tile([128, 2, LTW], F32, tag="sc", name=f"psv{st}")
            for kd in range(KD):
                nc.tensor.matmul(psv[:, 0, :FPC], xt[:, kd, ssl],
                                 wv_sb[:, kd, :],
                                 start=(kd == 0), stop=(kd == KD - 1))
            for h in range(HPC):
                off = 0 if h % 2 == 0 else 64
                if h < 2:
                    dst = Vaug8[:, q, p, h, off:off + 64]
                else:
                    dst = Vaug[:, st, h - 2, off:off + 64]
                nc.vector.tensor_copy(dst, psv[:, 0, DK * h:DK * (h + 1)])

        def emit_qt_half(lt, ft):
            lsl = slice(lt * LTW, (lt + 1) * LTW)
            psq = scp.tile([128, 1, LTW], F32, tag="sc", name=f"pq{lt}_{ft}")
            fsl = slice(ft * 128, (ft + 1) * 128)
            for kd in range(KD):
                nc.tensor.matmul(psq[:, 0, :], wq_sb[:, kd, fsl],
                                 xt[:, kd, lsl],
                                 start=(kd == 0), stop=(kd == KD - 1))
            nc.vector.scalar_tensor_tensor(
                QT[:, ft, lsl], psq[:, 0, :], 0.125,
                bq_sb[:, ft:ft + 1].to_broadcast((128, LTW)),
                mybir.AluOpType.mult, mybir.AluOpType.add)

        # ---------------- attention pipeline ----------------
        # Schraudolph tiles: ACT skips exp there; DVE computes masked-exp
        # u8 bits directly from PSUM. Disabled: the DVE stt sits on the
        # sc-ring critical path and stalls the exp chain more than it saves.
        SCH = {}                # st -> maskS8 slot
        mkbs = {}
        s8s = {}
        E8s = {}       # (lt, q) -> fp8 E block [128, h(4), stp(2), LTW]
        Ts_by_lt = {}

        def issue_mk(lt, st):
            # one DMA covers the (st, st+1) pair; only fire on even st
            if st % 2 == 1:
                return
            mkb = mpool.tile([128, 2, LTW // 2], U16, tag="mkb")
            nc.sync.dma_start(out=mkb[:], in_=maskB[lt][:, st:st + 2, :])
            mkbs[(lt, st // 2)] = mkb
            if st in SCH:
                s8 = mpool.tile([128, LTW], F16, tag="s8")
                nc.sync.dma_start(out=s8[:],
                                  in_=maskS8[lt][:, SCH[st], :])
                s8s[(lt, st)] = s8

        def emit_scores(lt, st):
            lsl = slice(lt * LTW, (lt + 1) * LTW)
            ssl = slice(st * STW, (st + 1) * STW)
            q, p = divmod(st, 2)
            if p == 0:
                E8s[(lt, q)] = epool.tile([128, 4, 2, LTW], F8, tag="E8",
                                          name=f"E8_{lt}_{q}")
            E8 = E8s[(lt, q)]
            sch = st in SCH
            for pair in range(2):
                sc = scp.tile([128, 2, LTW], F32, tag="sc")
                for i in range(2):
                    nc.tensor.matmul(
                        sc[:, i, :],
                        KT[64 * i:64 * (i + 1), pair, ssl],
                        QT[64 * i:64 * (i + 1), pair, lsl],
                        start=True, stop=True)
                hsl = slice(2 * pair, 2 * pair + 2)
                if sch:
                    s8 = s8s[(lt, st)]
                    nc.vector.scalar_tensor_tensor(
                        E8.bitcast(mybir.dt.uint8)[:, hsl, p, :], sc[:],
                        11.5416, s8[:, None, :].to_broadcast((128, 2, LTW)),
                        mybir.AluOpType.mult, mybir.AluOpType.add)
                else:
                    nc.scalar.activation(E8[:, hsl, p, :], sc[:], Exp)
            if sch:
                s8s.pop((lt, st))
            if p == 1:
                # one byte-AND masks all four heads and both planes
                mkb = mkbs.pop((lt, q))
                e8u = E8.bitcast(U16)
                nc.vector.tensor_tensor(
                    e8u[:], e8u[:],
                    mkb[:, None, :, :].to_broadcast((128, 4, 2, LTW // 2)),
                    mybir.AluOpType.bitwise_and)

        def emit_av(lt, st):
            q, p = divmod(st, 2)
            E8 = E8s[(lt, q)]
            Ts = Ts_by_lt[lt]
            for i in range(2):
                nc.tensor.matmul(Ts[2 + i][:], Vaug[:, st, i, :],
                                 E8[:, 2 + i, p, :],
                                 start=(st == 0), stop=(st == ST - 1))
            if p == 1:
                E8s.pop((lt, q))
                for h in range(2):
                    nc.tensor.matmul(Ts[h][:], Vaug8[:, q, :, h, :],
                                     E8[:, h, :, :],
                                     start=(q == 0), stop=(q == ST // 2 - 1),
                                     perf_mode=DRMODE)

        def emit_norm(lt):
            # reciprocal_approx_fast only works at partition base 0, so
            # route the row sums through lanes 0:64 in both parities.
            Ts = Ts_by_lt.pop(lt)
            for h in range(HPC):
                pair, i = divmod(h, 2)
                av_sl = slice(64 * i, 64 * (i + 1))        # av lanes
                rs_sl = slice(64 * (1 - i), 64 * (2 - i))  # row-sum lanes
                rb = rbpool.tile([128, LTW], F32)
                if i == 0:   # av 0:64, sums 64:128 -> move sums down first
                    nc.vector.tensor_copy(rb[64:128, :], Ts[h][rs_sl, :])
                    nc.sync.dma_start(out=rb[0:64, :], in_=rb[64:128, :])
                    nc.vector.reciprocal_approx_fast(out=rb[0:64, :],
                                                     in_=rb[0:64, :])
                else:        # sums 0:64 -> recip at base 0, then move up
                    nc.vector.reciprocal_approx_fast(out=rb[0:64, :],
                                                     in_=Ts[h][rs_sl, :])
                    nc.sync.dma_start(out=rb[64:128, :], in_=rb[0:64, :])
                nc.vector.tensor_mul(outTs[lt][av_sl, pair, :],
                                     Ts[h][av_sl, :], rb[av_sl, :])

        def emit_outproj_chunk(lt, c):
            lt8 = 4 * lt + c
            ps3 = scp.tile([128, 2, LTW], F32, tag="sc", name=f"ps3_{lt8}")
            for nf in range(2):
                nsl = slice(nf * 512, (nf + 1) * 512)
                for pair in range(2):
                    nc.tensor.matmul(
                        ps3[:, nf, :],
                        outTs[lt][:, pair, c * 128:(c + 1) * 128],
                        wo_sb[:, pair, nsl],
                        start=(pair == 0), stop=(pair == 1))
            ob = opool.tile([128, D], F16)
            if lt8 >= 12:  # tail chunks: ACT is idle after the last exp
                nc.scalar.copy(ob[:], ps3[:])
            else:          # Pool cannot read PSUM; DVE does the cast
                nc.vector.tensor_copy(ob[:], ps3[:])
            nc.sync.dma_start(out=out[:, lt8, :], in_=ob[:])

        # prologue: KT chunk 0 + QT(0) first, then the first three score
        # tiles so the exp pipeline starts early; remaining input DMAs and
        # projection chunks stream under it.
        emit_kt_chunk(0)
        emit_qt_half(0, 0)
        emit_qt_half(0, 1)
        seq = [(lt, st) for lt in range(LT) for st in range(ST)]
        issue_mk(0, 0)
        issue_mk(0, 1)
        issue_mk(0, 2)
        issue_mk(0, 3)
        emit_scores(0, 0)
        emit_scores(0, 1)
        emit_scores(0, 2)
        emit_scores(0, 3)
        nc.sync.dma_start(out=wv_sb[:], in_=wv[:])
        for q in range(1, 4):
            qsl = slice(q * LTW, (q + 1) * LTW)
            nc.sync.dma_start(out=xt[:, :, qsl], in_=xT[:, :, qsl])
        nc.sync.dma_start(out=wo_sb[:], in_=wo[:])
        emit_v_piece(0)
        emit_v_piece(1)
        emit_kt_half(1, 0)
        emit_kt_half(1, 1)

        # filler schedule: every PE lump outside scores/AV is cut into
        # <=1.7us pieces and spread one-per-iteration so the ACT exp chain
        # (the pacer, ~2.1us/tile) never starves on the 2-deep sc ring.
        fillers = {}

        def add_filler(idx, fn):
            fillers.setdefault(idx, []).append(fn)

        for st in range(2, ST):
            add_filler(st - 2, lambda st=st: emit_v_piece(st))
        add_filler(2, lambda: emit_kt_half(2, 0))
        add_filler(3, lambda: emit_kt_half(2, 1))
        add_filler(6, lambda: emit_kt_half(3, 0))
        add_filler(7, lambda: emit_kt_half(3, 1))
        for lt in range(LT - 1):
            add_filler(16 * lt + 8, lambda lt=lt: emit_qt_half(lt + 1, 0))
            add_filler(16 * lt + 11, lambda lt=lt: emit_qt_half(lt + 1, 1))
        for lt in range(1, LT):
            for j, stj in enumerate((2, 6, 10, 14)):
                add_filler(16 * lt + stj,
                           lambda lt=lt, j=j: emit_outproj_chunk(lt - 1, j))

        for idx, (lt, st) in enumerate(seq):
            if st == 0:
                Ts_by_lt[lt] = [
                    tp.tile([128, LTW], F32, tag=f"T{h}", name=f"T{h}_{lt}")
                    for h in range(HPC)]
            if idx + 4 < len(seq):
                issue_mk(*seq[idx + 4])
                emit_scores(*seq[idx + 4])
            emit_av(lt, st)
            for fn in fillers.get(idx, ()):
                fn()
            if st == ST - 1:
                emit_norm(lt)

        for c in range(4):
            emit_outproj_chunk(LT - 1, c)

    nc.compile()
    return nc


def _get_nc():
    global _CACHED_NC
    if _CACHED_NC is None:
        _CACHED_NC = _build()
    return _CACHED_NC


def _prep_core_inputs(c, x, mask, Wq, bq, Wk, bk, Wv, Wo):
    b, g = divmod(c, 4)
    cs = slice(g * FPC, (g + 1) * FPC)

    xT = np.ascontiguousarray(
        x[b].T.reshape(KD, 128, L).transpose(1, 0, 2)).astype(np.float16)
    wq_c = np.ascontiguousarray(
        Wq[:, cs].reshape(KD, 128, FPC).transpose(1, 0, 2)).astype(np.float16)
    wk_c = np.ascontiguousarray(
        Wk[:, cs].reshape(KD, 128, FPC).transpose(1, 0, 2)).astype(np.float16)
    wv_c = np.ascontiguousarray(
        Wv[:, cs].reshape(KD, 128, FPC).transpose(1, 0, 2)).astype(np.float16)
    wo_c = np.ascontiguousarray(
        Wo[cs, :].reshape(2, 128, D).transpose(1, 0, 2)).astype(np.float16)
    bq_c = np.ascontiguousarray(
        (bq[cs] * 0.125).reshape(2, 128).T).astype(np.float32)
    bk_c = np.ascontiguousarray(bk[cs].reshape(2, 128).T).astype(np.float32)
    # byte mask (0xFF keep / 0x00 drop) for the fp8 E blocks, u16-packed,
    # in [LT, 128(s within s-tile), ST, LTW//2] layout (st-pair = one DMA)
    mB = np.where(mask[b].T != 0, 0xFF, 0x00).astype(np.uint8)
    maskB = np.ascontiguousarray(
        mB.reshape(ST, 128, LT, LTW).transpose(2, 1, 0, 3))
    maskB = maskB.reshape(LT, 128, ST, LTW // 2, 2).view(np.uint16)[..., 0]
    # Schraudolph u8 bias for the sch tiles (st=6 -> slot 0, st=14 -> 1):
    # kept 55.44, masked -20000 (saturates to 0x00 = +0.0 in e4m3)
    mS = np.where(mask[b].T != 0, np.float16(55.44), np.float16(-20000.0))
    mS = mS.reshape(ST, 128, LT, LTW).transpose(2, 1, 0, 3)
    maskS8 = np.ascontiguousarray(mS[:, :, (6, 14), :])
    return {"xT": xT, "wq": wq_c, "wk": wk_c, "wv": wv_c, "wo": wo_c,
            "bq": bq_c, "bk": bk_c, "maskB": maskB, "maskS8": maskS8}


def kernel(x, mask, Wq, bq, Wk, bk, Wv, bv, Wo, bo):
    x = np.asarray(x, np.float32)
    mask = np.asarray(mask)
    Wq, bq = np.asarray(Wq, np.float32), np.asarray(bq, np.float32)
    Wk, bk = np.asarray(Wk, np.float32), np.asarray(bk, np.float32)
    Wv, bv = np.asarray(Wv, np.float32), np.asarray(bv, np.float32)
    Wo, bo = np.asarray(Wo, np.float32), np.asarray(bo, np.float32)

    nc = _get_nc()
    in_maps = [_prep_core_inputs(c, x, mask, Wq, bq, Wk, bk, Wv, Wo)
               for c in range(NCORES)]
    res = run_bass_kernel_spmd(nc, in_maps, list(range(NCORES)))

    const_vec = (bv @ Wo + bo).astype(np.float32)  # A rows sum to 1
    outs = []
    for b in range(B):
        acc = np.zeros((L, D), np.float32)
        for g in range(4):
            part = res.results[4 * b + g]["out"]  # [128, 16, 1024] fp16
            acc += part.transpose(1, 0, 2).reshape(L, D).astype(np.float32)
        acc += const_vec
        outs.append(acc)
    return np.stack(outs)



# revision 26
# speedup vs baseline: 1.0116x; 1.0116x over previous
"""Multi-head attention (B=2, L=S=2048, D=1024, H=16) on 8 Trainium2 cores.

Sharding: core c -> batch b = c // 4, head group g = c % 4 (4 heads per core).
W_Q/K/V column-sharded (256 cols per core), W_O row-sharded (256 rows per core);
the 4 partial outputs per batch are summed on the host (plus bias terms).

Per-core pipeline (all big tensors kept transposed so no on-device transposes):
  projections: QT = 0.125*(x Wq + bq)^T, KT = (x Wk + bk)^T (feature-major
    [256, L]); Vaug = [V_h | ones] per head (seq-major, fp16), V bias folded
    out on the host (softmax rows sum to 1 => + bv @ Wo + bo once).
  attention, per (l-tile 512, s-tile 128): S^T = KT^T QT (row-packed pairs of
    heads, K=64, the two 64-row matmuls co-execute in disjoint PE row
    halves); E = exp(S^T) * maskT; T_h += Vaug_h^T E accumulates the head
    output AND its softmax row-sums in one matmul (ones columns act as the
    reducer).

Scheduling: the 64 (lt, st) attention tiles form ONE flat software pipeline
with the AV accumulation running four s-tiles behind the score matmuls -
including across lt boundaries - so the PE never idle-waits on the
ACT exp -> DVE mask chain.  Idle waits re-throttle the PE clock to 1.2 GHz
(HAM activity monitor); in the unpipelined version ~36% of matmuls ran at
half clock.  The QT projection chain for lt+1 is emitted in two halves
mid-lt and each lt's out-projection is spread through the NEXT lt (PE
filler during the ACT-bound steady state); only lt3's out-projection runs
as a tail.  x arrives via one coarse strided DMA per l-quarter so KT/QT
chunk 0 finish after ~25% of x has landed and the exp pipeline starts
early.  ACT does exp only (+4 tail copies); QT/KT biases are DVE
scalar_tensor_tensor; softmax-sum lane swaps and output stores ride the
Sync DMA queue.

All matmul operands fp16 (1 cyc/row); PSUM fp32.  PSUM budget 8 banks =
scores 2x2 + T_h 4x1; projection and output-projection matmuls borrow the
same slots.
"""
from contextlib import ExitStack

import numpy as np

import concourse.bass as bass
import concourse.mybir as mybir
import concourse.tile as tile
from concourse import bacc
from concourse.bass_utils import run_bass_kernel_spmd

F16 = mybir.dt.float16
F32 = mybir.dt.float32
F8 = mybir.dt.float8e4
U16 = mybir.dt.uint16
DRMODE = mybir.MatmulPerfMode.DoubleRow

D = 1024          # d_model
H = 16            # heads
DK = 64           # head dim
B, L = 2, 2048
NCORES = 8
HPC = 4           # heads per core
FPC = HPC * DK    # features per core = 256
KD = D // 128     # 8 contraction subtiles for projections
LT, LTW = 4, 512  # l tiles
ST, STW = 16, 128  # s tiles
Exp = mybir.ActivationFunctionType.Exp

_CACHED_NC = None


def _build():
    nc = bacc.Bacc("TRN2", target_bir_lowering=False, debug=False,
                   num_devices=NCORES)
    xT = nc.declare_dram_parameter("xT", [128, KD, L], F16, isOutput=False)
    wq = nc.declare_dram_parameter("wq", [128, KD, FPC], F16, isOutput=False)
    wk = nc.declare_dram_parameter("wk", [128, KD, FPC], F16, isOutput=False)
    wv = nc.declare_dram_parameter("wv", [128, KD, FPC], F16, isOutput=False)
    wo = nc.declare_dram_parameter("wo", [128, 2, D], F16, isOutput=False)
    bq = nc.declare_dram_parameter("bq", [128, 2], F32, isOutput=False)
    bk = nc.declare_dram_parameter("bk", [128, 2], F32, isOutput=False)
    maskB = nc.declare_dram_parameter("maskB", [LT, 128, ST, LTW // 2], U16,
                                      isOutput=False)
    maskS8 = nc.declare_dram_parameter("maskS8", [LT, 128, 2, LTW], F16,
                                       isOutput=False)
    out = nc.declare_dram_parameter("out", [128, ST, D], F16, isOutput=True)

    with tile.TileContext(nc) as tc, ExitStack() as ctx:
        pool = ctx.enter_context(tc.tile_pool(name="pers", bufs=1))
        mpool = ctx.enter_context(tc.tile_pool(name="mpool", bufs=7))
        epool = ctx.enter_context(tc.tile_pool(name="epool", bufs=7))
        rbpool = ctx.enter_context(tc.tile_pool(name="rbpool", bufs=4))
        opool = ctx.enter_context(tc.tile_pool(name="opool", bufs=3))
        scp = ctx.enter_context(tc.tile_pool(name="scp", bufs=2, space="PSUM"))
        tp = ctx.enter_context(tc.tile_pool(name="tp", bufs=1, space="PSUM"))

        xt = pool.tile([128, KD, L], F16)
        wq_sb = pool.tile([128, KD, FPC], F16)
        wk_sb = pool.tile([128, KD, FPC], F16)
        wv_sb = pool.tile([128, KD, FPC], F16)
        wo_sb = pool.tile([128, 2, D], F16)
        bq_sb = pool.tile([128, 2], F32)
        bk_sb = pool.tile([128, 2], F32)
        # DMA order: l-quarter-major for xT (coarse strided DMAs - each
        # dma_start costs ~600ns of Sync queue time) so the KT-chunk-0 /
        # QT-chunk-0 chains (which contract over all KD but only touch
        # l 0:512) finish after ~25% of x has landed.
        nc.sync.dma_start(out=wk_sb[:], in_=wk[:])
        nc.sync.dma_start(out=xt[:, 0:4, 0:LTW], in_=xT[:, 0:4, 0:LTW])
        nc.sync.dma_start(out=xt[:, 4:KD, 0:LTW], in_=xT[:, 4:KD, 0:LTW])
        nc.sync.dma_start(out=wq_sb[:], in_=wq[:])
        nc.sync.dma_start(out=bk_sb[:], in_=bk[:])
        nc.sync.dma_start(out=bq_sb[:], in_=bq[:])

        QT = pool.tile([128, 2, L], F16)   # [feat(2x128), l]: Q^T * 0.125
        KT = pool.tile([128, 2, L], F16)
        # pair-1 heads (local h 2,3), fp16: [V_h | 1] / [1 | V_h] per parity
        # (memsets on DVE: gpsimd takes ~9us per tile and would gate the
        # first V copies and, via DVE program order, the kt1 bias-add)
        Vaug = pool.tile([128, ST, 2, 128], F16)
        nc.vector.memset(Vaug[:], 1.0)
        # pair-0 heads (local h 0,1), fp8 for DoubleRow AV:
        # [s, stq(8), stp(2), h(2), 128]
        Vaug8 = pool.tile([128, ST // 2, 2, 2, 128], F8)
        nc.vector.memset(Vaug8[:], 1.0)
        outTs = [pool.tile([128, 2, LTW], F16, name=f"outT{i}")
                 for i in range(LT)]

        def emit_kt_chunk(c):
            lsl = slice(c * LTW, (c + 1) * LTW)
            ps = scp.tile([128, 2, LTW], F32, tag="sc", name=f"pk{c}")
            for ft in range(2):
                fsl = slice(ft * 128, (ft + 1) * 128)
                for kd in range(KD):
                    nc.tensor.matmul(ps[:, ft, :], wk_sb[:, kd, fsl],
                                     xt[:, kd, lsl],
                                     start=(kd == 0), stop=(kd == KD - 1))
                nc.vector.scalar_tensor_tensor(
                    KT[:, ft, lsl], ps[:, ft, :], 1.0,
                    bk_sb[:, ft:ft + 1].to_broadcast((128, LTW)),
                    mybir.AluOpType.mult, mybir.AluOpType.add)

        def emit_kt_half(c, ft):
            lsl = slice(c * LTW, (c + 1) * LTW)
            psk = scp.tile([128, 1, LTW], F32, tag="sc", name=f"pk{c}_{ft}")
            fsl = slice(ft * 128, (ft + 1) * 128)
            for kd in range(KD):
                nc.tensor.matmul(psk[:, 0, :], wk_sb[:, kd, fsl],
                                 xt[:, kd, lsl],
                                 start=(kd == 0), stop=(kd == KD - 1))
            nc.vector.scalar_tensor_tensor(
                KT[:, ft, lsl], psk[:, 0, :], 1.0,
                bk_sb[:, ft:ft + 1].to_broadcast((128, LTW)),
                mybir.AluOpType.mult, mybir.AluOpType.add)

        def emit_v_piece(st):
            ssl = slice(st * STW, (st + 1) * STW)
            q, p = divmod(st, 2)
            # rides the sc ring (a filler like kt/qt); T banks are taken
            # by the running lt's accumulators
            psv = scp.tile([128, 2, LTW], F32, tag="sc", name=f"psv{st}")
            for kd in range(KD):
                nc.tensor.matmul(psv[:, 0, :FPC], xt[:, kd, ssl],
                                 wv_sb[:, kd, :],
                                 start=(kd == 0), stop=(kd == KD - 1))
            for h in range(HPC):
                off = 0 if h % 2 == 0 else 64
                if h < 2:
                    dst = Vaug8[:, q, p, h, off:off + 64]
                else:
                    dst = Vaug[:, st, h - 2, off:off + 64]
                nc.vector.tensor_copy(dst, psv[:, 0, DK * h:DK * (h + 1)])

        def emit_qt_half(lt, ft):
            lsl = slice(lt * LTW, (lt + 1) * LTW)
            psq = scp.tile([128, 1, LTW], F32, tag="sc", name=f"pq{lt}_{ft}")
            fsl = slice(ft * 128, (ft + 1) * 128)
            for kd in range(KD):
                nc.tensor.matmul(psq[:, 0, :], wq_sb[:, kd, fsl],
                                 xt[:, kd, lsl],
                                 start=(kd == 0), stop=(kd == KD - 1))
            nc.vector.scalar_tensor_tensor(
                QT[:, ft, lsl], psq[:, 0, :], 0.125,
                bq_sb[:, ft:ft + 1].to_broadcast((128, LTW)),
                mybir.AluOpType.mult, mybir.AluOpType.add)

        # ---------------- attention pipeline ----------------
        # Schraudolph tiles: ACT skips exp there; DVE computes masked-exp
        # u8 bits directly from PSUM. Disabled: the DVE stt sits on the
        # sc-ring critical path and stalls the exp chain more than it saves.
        SCH = {}                # st -> maskS8 slot
        mkbs = {}
        s8s = {}
        E8s = {}       # (lt, q) -> fp8 E block [128, h(4), stp(2), LTW]
        Ts_by_lt = {}

        def issue_mk(lt, st):
            # one DMA covers the (st, st+1) pair; only fire on even st
            if st % 2 == 1:
                return
            mkb = mpool.tile([128, 2, LTW // 2], U16, tag="mkb")
            nc.sync.dma_start(out=mkb[:], in_=maskB[lt][:, st:st + 2, :])
            mkbs[(lt, st // 2)] = mkb
            if st in SCH:
                s8 = mpool.tile([128, LTW], F16, tag="s8")
                nc.sync.dma_start(out=s8[:],
                                  in_=maskS8[lt][:, SCH[st], :])
                s8s[(lt, st)] = s8

        def emit_scores(lt, st, auto_and=True):
            lsl = slice(lt * LTW, (lt + 1) * LTW)
            ssl = slice(st * STW, (st + 1) * STW)
            q, p = divmod(st, 2)
            if p == 0:
                E8s[(lt, q)] = epool.tile([128, 4, 2, LTW], F8, tag="E8",
                                          name=f"E8_{lt}_{q}")
            E8 = E8s[(lt, q)]
            sch = st in SCH
            for pair in range(2):
                sc = scp.tile([128, 2, LTW], F32, tag="sc")
                for i in range(2):
                    nc.tensor.matmul(
                        sc[:, i, :],
                        KT[64 * i:64 * (i + 1), pair, ssl],
                        QT[64 * i:64 * (i + 1), pair, lsl],
                        start=True, stop=True)
                hsl = slice(2 * pair, 2 * pair + 2)
                if sch:
                    s8 = s8s[(lt, st)]
                    nc.vector.scalar_tensor_tensor(
                        E8.bitcast(mybir.dt.uint8)[:, hsl, p, :], sc[:],
                        11.5416, s8[:, None, :].to_broadcast((128, 2, LTW)),
                        mybir.AluOpType.mult, mybir.AluOpType.add)
                else:
                    nc.scalar.activation(E8[:, hsl, p, :], sc[:], Exp)
            if sch:
                s8s.pop((lt, st))
            if p == 1 and auto_and:
                emit_and(lt, q)

        def emit_and(lt, q):
            # one byte-AND masks all four heads and both planes
            mkb = mkbs.pop((lt, q))
            E8 = E8s[(lt, q)]
            e8u = E8.bitcast(U16)
            nc.vector.tensor_tensor(
                e8u[:], e8u[:],
                mkb[:, None, :, :].to_broadcast((128, 4, 2, LTW // 2)),
                mybir.AluOpType.bitwise_and)

        def emit_av(lt, st):
            q, p = divmod(st, 2)
            E8 = E8s[(lt, q)]
            Ts = Ts_by_lt[lt]
            if p == 1:
                # pair-0 DoubleRow first: at st==15 this lets norm(h0,h1)
                # start while the pair-1 matmuls still run
                E8s.pop((lt, q))
                for h in range(2):
                    nc.tensor.matmul(Ts[h][:], Vaug8[:, q, :, h, :],
                                     E8[:, h, :, :],
                                     start=(q == 0), stop=(q == ST // 2 - 1),
                                     perf_mode=DRMODE)
            for i in range(2):
                nc.tensor.matmul(Ts[2 + i][:], Vaug[:, st, i, :],
                                 E8[:, 2 + i, p, :],
                                 start=(st == 0), stop=(st == ST - 1))

        def emit_norm(lt):
            # reciprocal_approx_fast only works at partition base 0, so
            # route the row sums through lanes 0:64 in both parities.
            Ts = Ts_by_lt.pop(lt)
            for h in range(HPC):
                pair, i = divmod(h, 2)
                av_sl = slice(64 * i, 64 * (i + 1))        # av lanes
                rs_sl = slice(64 * (1 - i), 64 * (2 - i))  # row-sum lanes
                rb = rbpool.tile([128, LTW], F32)
                if i == 0:   # av 0:64, sums 64:128 -> move sums down first
                    nc.vector.tensor_copy(rb[64:128, :], Ts[h][rs_sl, :])
                    nc.sync.dma_start(out=rb[0:64, :], in_=rb[64:128, :])
                    nc.vector.reciprocal_approx_fast(out=rb[0:64, :],
                                                     in_=rb[0:64, :])
                else:        # sums 0:64 -> recip at base 0, then move up
                    nc.vector.reciprocal_approx_fast(out=rb[0:64, :],
                                                     in_=Ts[h][rs_sl, :])
                    nc.sync.dma_start(out=rb[64:128, :], in_=rb[0:64, :])
                nc.vector.tensor_mul(outTs[lt][av_sl, pair, :],
                                     Ts[h][av_sl, :], rb[av_sl, :])

        def emit_outproj_chunk(lt, c):
            lt8 = 4 * lt + c
            ps3 = scp.tile([128, 2, LTW], F32, tag="sc", name=f"ps3_{lt8}")
            for nf in range(2):
                nsl = slice(nf * 512, (nf + 1) * 512)
                for pair in range(2):
                    nc.tensor.matmul(
                        ps3[:, nf, :],
                        outTs[lt][:, pair, c * 128:(c + 1) * 128],
                        wo_sb[:, pair, nsl],
                        start=(pair == 0), stop=(pair == 1))
            ob = opool.tile([128, D], F16)
            if lt8 >= 12:  # tail chunks: ACT is idle after the last exp
                nc.scalar.copy(ob[:], ps3[:])
            else:          # Pool cannot read PSUM; DVE does the cast
                nc.vector.tensor_copy(ob[:], ps3[:])
            nc.sync.dma_start(out=out[:, lt8, :], in_=ob[:])

        # PE warmup: dummy matmuls ramp the PE clock out of the low pstate
        # while the first input DMAs are still in flight.
        wu = pool.tile([128, LTW], F16)
        nc.gpsimd.memset(wu[:], 0.0)  # gpsimd is otherwise idle at t0
        psw = scp.tile([128, 2, LTW], F32, tag="sc", name="psw")
        for i in range(12):
            nc.tensor.matmul(psw[:, 0, :], wu[:, 0:128], wu[:],
                             start=(i == 0), stop=(i == 11))

        # prologue: KT chunk 0 + QT(0) first, then the first four score
        # tiles so the exp pipeline starts early; remaining input DMAs and
        # the first V/KT pieces stream under it. The q0/q1 byte-ANDs are
        # deferred past the V/KT DVE work (DVE executes in order; an early
        # AND waiting on exp would block the kt1 bias-add and thereby
        # stall scores(0,4)).
        nc.sync.dma_start(out=wv_sb[:], in_=wv[:])
        for q in range(1, 4):
            qsl = slice(q * LTW, (q + 1) * LTW)
            nc.sync.dma_start(out=xt[:, :, qsl], in_=xT[:, :, qsl])
        nc.sync.dma_start(out=wo_sb[:], in_=wo[:])
        emit_kt_chunk(0)
        emit_qt_half(0, 0)
        emit_qt_half(0, 1)
        seq = [(lt, st) for lt in range(LT) for st in range(ST)]
        issue_mk(0, 0)
        issue_mk(0, 1)
        issue_mk(0, 2)
        issue_mk(0, 3)
        emit_scores(0, 0)
        emit_scores(0, 1, auto_and=False)
        emit_v_piece(0)
        emit_v_piece(1)
        emit_kt_half(1, 0)
        emit_kt_half(1, 1)
        emit_scores(0, 2)
        emit_scores(0, 3, auto_and=False)
        emit_and(0, 0)
        emit_and(0, 1)

        # filler schedule: every PE lump outside scores/AV is cut into
        # <=1.7us pieces and spread one-per-iteration so the ACT exp chain
        # (the pacer, ~2.1us/tile) never starves on the 2-deep sc ring.
        fillers = {}

        def add_filler(idx, fn):
            fillers.setdefault(idx, []).append(fn)

        for st in range(2, ST):
            add_filler(st - 2, lambda st=st: emit_v_piece(st))
        add_filler(2, lambda: emit_kt_half(2, 0))
        add_filler(3, lambda: emit_kt_half(2, 1))
        add_filler(6, lambda: emit_kt_half(3, 0))
        add_filler(7, lambda: emit_kt_half(3, 1))
        for lt in range(LT - 1):
            add_filler(16 * lt + 8, lambda lt=lt: emit_qt_half(lt + 1, 0))
            add_filler(16 * lt + 11, lambda lt=lt: emit_qt_half(lt + 1, 1))
        for lt in range(1, LT):
            for j, stj in enumerate((2, 6, 10, 14)):
                add_filler(16 * lt + stj,
                           lambda lt=lt, j=j: emit_outproj_chunk(lt - 1, j))

        for idx, (lt, st) in enumerate(seq):
            if st == 0:
                Ts_by_lt[lt] = [
                    tp.tile([128, LTW], F32, tag=f"T{h}", name=f"T{h}_{lt}")
                    for h in range(HPC)]
            if idx + 4 < len(seq):
                issue_mk(*seq[idx + 4])
                emit_scores(*seq[idx + 4])
            emit_av(lt, st)
            for fn in fillers.get(idx, ()):
                fn()
            if st == ST - 1:
                emit_norm(lt)

        for c in range(4):
            emit_outproj_chunk(LT - 1, c)

    nc.compile()
    return nc


def _get_nc():
    global _CACHED_NC
    if _CACHED_NC is None:
        _CACHED_NC = _build()
    return _CACHED_NC


def _prep_core_inputs(c, x, mask, Wq, bq, Wk, bk, Wv, Wo):
    b, g = divmod(c, 4)
    cs = slice(g * FPC, (g + 1) * FPC)

    xT = np.ascontiguousarray(
        x[b].T.reshape(KD, 128, L).transpose(1, 0, 2)).astype(np.float16)
    wq_c = np.ascontiguousarray(
        Wq[:, cs].reshape(KD, 128, FPC).transpose(1, 0, 2)).astype(np.float16)
    wk_c = np.ascontiguousarray(
        Wk[:, cs].reshape(KD, 128, FPC).transpose(1, 0, 2)).astype(np.float16)
    wv_c = np.ascontiguousarray(
        Wv[:, cs].reshape(KD, 128, FPC).transpose(1, 0, 2)).astype(np.float16)
    wo_c = np.ascontiguousarray(
        Wo[cs, :].reshape(2, 128, D).transpose(1, 0, 2)).astype(np.float16)
    bq_c = np.ascontiguousarray(
        (bq[cs] * 0.125).reshape(2, 128).T).astype(np.float32)
    bk_c = np.ascontiguousarray(bk[cs].reshape(2, 128).T).astype(np.float32)
    # byte mask (0xFF keep / 0x00 drop) for the fp8 E blocks, u16-packed,
    # in [LT, 128(s within s-tile), ST, LTW//2] layout (st-pair = one DMA)
    mB = np.where(mask[b].T != 0, 0xFF, 0x00).astype(np.uint8)
    maskB = np.ascontiguousarray(
        mB.reshape(ST, 128, LT, LTW).transpose(2, 1, 0, 3))
    maskB = maskB.reshape(LT, 128, ST, LTW // 2, 2).view(np.uint16)[..., 0]
    # Schraudolph u8 bias for the sch tiles (st=6 -> slot 0, st=14 -> 1):
    # kept 55.44, masked -20000 (saturates to 0x00 = +0.0 in e4m3)
    mS = np.where(mask[b].T != 0, np.float16(55.44), np.float16(-20000.0))
    mS = mS.reshape(ST, 128, LT, LTW).transpose(2, 1, 0, 3)
    maskS8 = np.ascontiguousarray(mS[:, :, (6, 14), :])
    return {"xT": xT, "wq": wq_c, "wk": wk_c, "wv": wv_c, "wo": wo_c,
            "bq": bq_c, "bk": bk_c, "maskB": maskB, "maskS8": maskS8}


def kernel(x, mask, Wq, bq, Wk, bk, Wv, bv, Wo, bo):
    x = np.asarray(x, np.float32)
    mask = np.asarray(mask)
    Wq, bq = np.asarray(Wq, np.float32), np.asarray(bq, np.float32)
    Wk, bk = np.asarray(Wk, np.float32), np.asarray(bk, np.float32)
    Wv, bv = np.asarray(Wv, np.float32), np.asarray(bv, np.float32)
    Wo, bo = np.asarray(Wo, np.float32), np.asarray(bo, np.float32)

    nc = _get_nc()
    in_maps = [_prep_core_inputs(c, x, mask, Wq, bq, Wk, bk, Wv, Wo)
               for c in range(NCORES)]
    res = run_bass_kernel_spmd(nc, in_maps, list(range(NCORES)))

    const_vec = (bv @ Wo + bo).astype(np.float32)  # A rows sum to 1
    outs = []
    for b in range(B):
        acc = np.zeros((L, D), np.float32)
        for g in range(4):
            part = res.results[4 * b + g]["out"]  # [128, 16, 1024] fp16
            acc += part.transpose(1, 0, 2).reshape(L, D).astype(np.float32)
        acc += const_vec
        outs.append(acc)
    return np.stack(outs)

